# revision 27
# baseline (speedup 1.0000x reference)
"""Trainium2 Bass kernel for MipRayMarcher2 (volume-rendering ray marcher).

Input (full): colors [4,4096,96,32] f32, densities [4,4096,96,1] f32,
depths [4,4096,96,1] f32.  Output: the reference's 5-tuple
(composite_rgb, composite_depth, weights, alpha, weight_bg).

Strategy: shard batch*rays = 16384 rays over 8 cores (2048 rays/core),
rays on SBUF partitions.  Per core, per partition p, rays 16p..16p+15
(16 "groups" g, ray = 16p + g).

Math (per ray, S=96 samples, EPS=1e-10):
  dd[s]    = softplus(0.5*(den[s]+den[s+1]) - 1) * (dep[s+1]-dep[s])   s<95
  em[s]    = exp(-dd[s]);  alpha[s] = 1 - em[s]
  trans    = cumprod([1, em+EPS])          (exact, via tensor_tensor_scan)
  weights[s] = alpha[s] * trans[s]
  weight_bg  = trans[95]
  rgbsum[c]  = sum_{s'=0..95} (wext[s'] + wext[s'+1]) * colors[s',c]
               where wext = [0, w0..w94, 0]       (== 2*sum_s w*colors_mid)
  composite_rgb = rgbsum - 1                       (host)
  composite_depth = (0.5*sum_s w*(dep[s]+dep[s+1])) / sum_s w, nan->inf,
                    clipped to [min(depths), max(depths)]  (host, tiny)

The big colors reduction runs as: DVE broadcast-multiply colors*v2 ->
float32r, then TensorE copy-accumulate matmuls (identity stationary,
fp32r moving, FD=256) summing 12 s-blocks into PSUM, then a small
strided DVE reduce folds the remaining 8 s-congruence-classes.
"""
import sys

if "/opt/trn_rl_repo" not in sys.path:
    sys.path.insert(0, "/opt/trn_rl_repo")

from contextlib import ExitStack

import numpy as np

import concourse.bass as bass  # noqa: F401  (engine types referenced via nc)
import concourse.tile as tile
from concourse import bacc, mybir
from concourse.bass_utils import run_bass_kernel_spmd

# problem constants (hardcoded per harness contract)
B, R, S, C = 4, 4096, 96, 32
N_CORES = 8
RAYS = B * R                       # 16384
RPC = RAYS // N_CORES              # 2048 rays per core
P = 128                            # SBUF partitions
G = RPC // P                       # 16 ray-groups per core
F = S * C                          # 3072 colors floats per ray
EPS = 1e-10
FD = 256                           # matmul moving free dim (>=256 for fp32r full rate)
NBLK = F // FD                     # 12 accumulation blocks
SL = FD // C                       # 8 s-congruence classes left after PSUM accum

f32 = mybir.dt.float32
f32r = mybir.dt.float32r
ALU = mybir.AluOpType
_COLORS_DMA_SPLIT = "alt"  # "sp" = all colors DMAs on SP ring
_MUL_SPLIT = 0  # channels of the big mul offloaded to gpsimd
_DEFER_FOLD = True
_CBUFS = 6
_WCBUFS = 2
ACTF = mybir.ActivationFunctionType


def _build(loop_k: int = 1, ablate: str = ""):
    """ablate: '' full kernel; 'dmaonly' loads colors but skips phase-1
    compute; 'nodma' runs phase-1 compute on a single preloaded colors
    tile (no per-group colors DMA)."""
    nc = bacc.Bacc()

    colors = nc.declare_dram_parameter("colors", [RPC, F], f32, isOutput=False)
    dens = nc.declare_dram_parameter("dens", [RPC, S], f32, isOutput=False)
    dep = nc.declare_dram_parameter("dep", [RPC, S], f32, isOutput=False)
    ident = nc.declare_dram_parameter("ident", [P, P], f32, isOutput=False)

    weights_o = nc.declare_dram_parameter("weights_o", [RPC, S - 1], f32, isOutput=True)
    alpha_o = nc.declare_dram_parameter("alpha_o", [RPC, S - 1], f32, isOutput=True)
    rgbsum_o = nc.declare_dram_parameter("rgbsum_o", [RPC, C], f32, isOutput=True)
    cdnum2_o = nc.declare_dram_parameter("cdnum2_o", [RPC, 1], f32, isOutput=True)
    wt_o = nc.declare_dram_parameter("wt_o", [RPC, 1], f32, isOutput=True)
    wbg_o = nc.declare_dram_parameter("wbg_o", [RPC, 1], f32, isOutput=True)

    with ExitStack() as ctx:
        tc = ctx.enter_context(tile.TileContext(nc))
        small = ctx.enter_context(tc.tile_pool(name="small", bufs=1))
        cpool = ctx.enter_context(tc.tile_pool(name="colors", bufs=_CBUFS))
        wcpool = ctx.enter_context(tc.tile_pool(name="wc", bufs=_WCBUFS))
        rgbpool = ctx.enter_context(tc.tile_pool(name="rgb", bufs=2))
        psum = ctx.enter_context(tc.tile_pool(name="psum", bufs=2, space="PSUM"))

        if loop_k > 1:
            loop_cm = tc.For_i(0, loop_k, 1)
            loop_cm.__enter__()
            ctx.callback(lambda: loop_cm.__exit__(None, None, None))

        # ---------------- phase 0: per-ray small tensors (all 2048 rays) ----
        # small inputs via SWDGE (gpsimd) to keep the HWDGE rings free for
        # the colors stream
        dens_t = small.tile([P, G * S], f32)
        nc.sync.dma_start(dens_t[:], dens[:].rearrange("(p g) s -> p (g s)", g=G))
        dep_t = small.tile([P, G * S], f32)
        nc.scalar.dma_start(dep_t[:], dep[:].rearrange("(p g) s -> p (g s)", g=G))

        # identity early too (tiny)
        ident_sb = small.tile([P, P], f32)
        nc.sync.dma_start(ident_sb[:], ident[:])
        ident_r = small.tile([P, P], f32r)
        nc.vector.tensor_copy(ident_r[:], ident_sb[:])

        # pre-issue the first _CBUFS colors DMAs AFTER the tiny inputs so
        # they don't starve phase 0; both HWDGE rings stream from ~2us
        colors_pg = colors[:].rearrange("(p g) f -> p g f", g=G)
        pre_tiles = {}
        if ablate != "nodma":
            npre = min(_CBUFS, G)
            for g in range(npre):
                dma_eng = nc.sync if _COLORS_DMA_SPLIT == "sp" else (
                    nc.sync if g % 2 == 0 else nc.scalar)
                t = cpool.tile([P, F], f32, tag="col_t")
                dma_eng.dma_start(t[:], colors_pg[:, g, :])
                pre_tiles[g] = t

        dens_v = dens_t[:].rearrange("p (g s) -> p g s", s=S)
        dep_v = dep_t[:].rearrange("p (g s) -> p g s", s=S)

        # bias constants for ACT ops (must be APs)
        minus1_t = small.tile([P, 1], f32)
        nc.gpsimd.memset(minus1_t[:], -1.0)
        eps_t = small.tile([P, 1], f32)
        nc.gpsimd.memset(eps_t[:], EPS)

        # dsum = den[s] + den[s+1]           (gpsimd)
        dsum_t = small.tile([P, G * (S - 1)], f32)
        dsum_v = dsum_t[:].rearrange("p (g s) -> p g s", s=S - 1)
        nc.vector.tensor_tensor(
            dsum_v, dens_v[:, :, 0:S - 1], dens_v[:, :, 1:S], op=ALU.add
        )
        # sp = softplus(0.5*dsum - 1) = ln(1 + exp(0.5*dsum - 1))   (ACT)
        # (no softplus in the ACT tables; exp+ln share one table)
        sp_t = small.tile([P, G * (S - 1)], f32)
        nc.scalar.activation(
            sp_t[:], dsum_t[:], ACTF.Exp, bias=minus1_t[:], scale=0.5
        )
        nc.scalar.activation(sp_t[:], sp_t[:], ACTF.Ln, bias=1.0)

        # delta = dep[s+1] - dep[s]          (DVE)
        delta_t = small.tile([P, G * (S - 1)], f32)
        delta_v = delta_t[:].rearrange("p (g s) -> p g s", s=S - 1)
        nc.vector.tensor_tensor(
            delta_v, dep_v[:, :, 1:S], dep_v[:, :, 0:S - 1], op=ALU.subtract
        )
        # dd = sp * delta                    (DVE)
        dd_t = small.tile([P, G * (S - 1)], f32)
        nc.vector.tensor_tensor(dd_t[:], sp_t[:], delta_t[:], op=ALU.mult)

        # em = exp(-dd)                      (ACT)  [slot reuse: dsum]
        em_t = small.tile([P, G * (S - 1)], f32, tag="dsum_t")
        nc.scalar.activation(em_t[:], dd_t[:], ACTF.Exp, scale=-1.0)
        # emeps = em + EPS                   (DVE TS) [slot reuse: delta]
        emeps_t = small.tile([P, G * (S - 1)], f32, tag="delta_t")
        nc.vector.tensor_scalar_add(emeps_t[:], em_t[:], EPS)
        # alpha = 1 - em = em*-1 + 1         (DVE TS) [slot reuse: sp]
        alpha_t = small.tile([P, G * (S - 1)], f32, tag="sp_t")
        nc.vector.tensor_scalar(
            alpha_t[:], em_t[:], -1.0, 1.0, op0=ALU.mult, op1=ALU.add
        )
        nc.gpsimd.dma_start(
            alpha_o[:].rearrange("(p g) s -> p (g s)", g=G), alpha_t[:]
        )

        # trans = cumprod([1, emeps])  per group scan (DVE)
        trans_t = small.tile([P, G * S], f32)
        trans_v = trans_t[:].rearrange("p (g s) -> p g s", s=S)
        nc.gpsimd.memset(trans_v[:, :, 0:1], 1.0)
        emeps_v = emeps_t[:].rearrange("p (g s) -> p g s", s=S - 1)
        for g in range(G):
            nc.vector.tensor_tensor_scan(
                trans_v[:, g, 1:S], emeps_v[:, g, :], emeps_v[:, g, :],
                1.0, op0=ALU.mult, op1=ALU.bypass,
            )

        # wext = [0, w0..w94, 0] per group; weights = alpha*trans[0:95]
        wext_t = small.tile([P, G * (S + 1)], f32)
        wext_v = wext_t[:].rearrange("p (g s) -> p g s", s=S + 1)
        nc.gpsimd.memset(wext_v[:, :, 0:1], 0.0)
        nc.gpsimd.memset(wext_v[:, :, S:S + 1], 0.0)
        alpha_v = alpha_t[:].rearrange("p (g s) -> p g s", s=S - 1)
        nc.vector.tensor_tensor(
            wext_v[:, :, 1:S], alpha_v, trans_v[:, :, 0:S - 1], op=ALU.mult
        )
        # contiguous staging copy (gpsimd; keeps the ACT queue free so its
        # HWDGE ring can issue colors DMAs early), then one contiguous DMA
        wcopy_t = small.tile([P, G * (S - 1)], f32, tag="dsum_t")
        nc.gpsimd.tensor_copy(
            wcopy_t[:].rearrange("p (g s) -> p g s", s=S - 1), wext_v[:, :, 1:S]
        )
        nc.gpsimd.dma_start(
            weights_o[:].rearrange("(p g) s -> p (g s)", g=G), wcopy_t[:]
        )
        # weight_bg = trans[95] -> contiguous staging, then DMA
        wbg_t = small.tile([P, G], f32)
        nc.vector.tensor_copy(wbg_t[:], trans_v[:, :, S - 1:S].squeeze(2))
        nc.scalar.dma_start(
            wbg_o[:].rearrange("(p g) s -> p g s", g=G), wbg_t[:].unsqueeze(2)
        )

        # v2[s'] = wext[s'] + wext[s'+1]     (DVE)
        v2_t = small.tile([P, G * S], f32)
        v2_v = v2_t[:].rearrange("p (g s) -> p g s", s=S)
        nc.vector.tensor_tensor(
            v2_v, wext_v[:, :, 0:S], wext_v[:, :, 1:S + 1], op=ALU.add
        )

        # dm2 = dep[s] + dep[s+1]            (gpsimd)
        dm2_t = small.tile([P, G * (S - 1)], f32)
        dm2_v = dm2_t[:].rearrange("p (g s) -> p g s", s=S - 1)
        nc.gpsimd.tensor_tensor(
            dm2_v, dep_v[:, :, 0:S - 1], dep_v[:, :, 1:S], op=ALU.add
        )
        # wd = weights * dm2                 (DVE)  [slot reuse: dd]
        wd_t = small.tile([P, G * (S - 1)], f32, tag="dd_t")
        wd_v = wd_t[:].rearrange("p (g s) -> p g s", s=S - 1)
        nc.vector.tensor_tensor(wd_v, wext_v[:, :, 1:S], dm2_v, op=ALU.mult)
        # cdnum2[g] = sum_s wd               (DVE strided reduce, innermost s)
        cdnum2_t = small.tile([P, G], f32)
        nc.vector.tensor_reduce(cdnum2_t[:], wd_v, axis=mybir.AxisListType.X, op=ALU.add)
        nc.scalar.dma_start(
            cdnum2_o[:].rearrange("(p g) s -> p g s", g=G), cdnum2_t[:].unsqueeze(2)
        )
        # wt[g] = sum_s weights
        wt_t = small.tile([P, G], f32)
        nc.vector.tensor_reduce(
            wt_t[:], wext_v[:, :, 1:S], axis=mybir.AxisListType.X, op=ALU.add
        )
        nc.scalar.dma_start(
            wt_o[:].rearrange("(p g) s -> p g s", g=G), wt_t[:].unsqueeze(2)
        )


        # ---------------- phase 1: stream colors, weighted reduce ----------
        # colors DMAs alternate between the two HWDGE rings (SP and ACT);
        # rgb results accumulate in one tile, flushed in a single DMA.
        colors_g = colors[:].rearrange("(p g) f -> p g f", g=G)
        rgb_all = rgbpool.tile([P, G * C], f32)
        pending = None
        pre_t = None
        if ablate == "nodma":
            pre_t = cpool.tile([P, F], f32, tag="pre")
            nc.sync.dma_start(pre_t[:], colors_g[:, 0, :])
        for g in range(G):
            dma_eng = nc.sync if _COLORS_DMA_SPLIT == "sp" else (
                nc.sync if g % 2 == 0 else nc.scalar
            )
            if ablate == "nodma":
                col_t = pre_t
            elif g in pre_tiles:
                col_t = pre_tiles[g]
            else:
                col_t = cpool.tile([P, F], f32, tag="col_t")
                dma_eng.dma_start(col_t[:], colors_g[:, g, :])

            if ablate == "dmaonly":
                # consume the tile so the DMA isn't dead-code-eliminated
                nc.vector.tensor_copy(
                    rgb_all[:, g * C:(g + 1) * C], col_t[:, 0:C]
                )
                continue

            wc_t = wcpool.tile([P, F], f32r)
            wc_v3 = wc_t[:].rearrange("p (s c) -> p s c", c=C)
            col_v3 = col_t[:].rearrange("p (s c) -> p s c", c=C)
            if _MUL_SPLIT > 0:
                cs = _MUL_SPLIT  # channels handled by gpsimd
                v2_bd = v2_t[:, g * S:(g + 1) * S].unsqueeze(2).broadcast_to(
                    [P, S, C - cs])
                nc.vector.tensor_tensor(
                    wc_v3[:, :, 0:C - cs], col_v3[:, :, 0:C - cs], v2_bd,
                    op=ALU.mult,
                )
                v2_bg = v2_t[:, g * S:(g + 1) * S].unsqueeze(2).broadcast_to(
                    [P, S, cs])
                nc.gpsimd.tensor_tensor(
                    wc_v3[:, :, C - cs:C], col_v3[:, :, C - cs:C], v2_bg,
                    op=ALU.mult,
                )
            else:
                v2_b = v2_t[:, g * S:(g + 1) * S].unsqueeze(2).broadcast_to([P, S, C])
                nc.vector.tensor_tensor(wc_v3, col_v3, v2_b, op=ALU.mult)

            if ablate == "nomm":
                nc.vector.tensor_copy(rgb_all[:, g * C:(g + 1) * C], wc_t[:, 0:C].bitcast(f32))
                continue

            ps_t = psum.tile([P, FD], f32)
            for b in range(NBLK):
                nc.tensor.matmul(
                    ps_t[:], ident_r[:], wc_t[:, b * FD:(b + 1) * FD],
                    start=(b == 0), stop=(b == NBLK - 1),
                )

            # defer the psum fold by one group: DVE is strict FIFO, so
            # folding immediately would stall it waiting on the matmuls
            if _DEFER_FOLD:
                if pending is not None:
                    pps, pg = pending
                    nc.vector.tensor_reduce(
                        rgb_all[:, pg * C:(pg + 1) * C],
                        pps[:].rearrange("p (sl c) -> p c sl", c=C),
                        axis=mybir.AxisListType.X, op=ALU.add,
                    )
                pending = (ps_t, g)
            else:
                nc.vector.tensor_reduce(
                    rgb_all[:, g * C:(g + 1) * C],
                    ps_t[:].rearrange("p (sl c) -> p c sl", c=C),
                    axis=mybir.AxisListType.X, op=ALU.add,
                )
        if pending is not None:
            pps, pg = pending
            nc.vector.tensor_reduce(
                rgb_all[:, pg * C:(pg + 1) * C],
                pps[:].rearrange("p (sl c) -> p c sl", c=C),
                axis=mybir.AxisListType.X, op=ALU.add,
            )
        nc.sync.dma_start(
            rgbsum_o[:].rearrange("(p g) c -> p (g c)", g=G), rgb_all[:]
        )

    nc.finalize()
    return nc


_NC_CACHE = None


def _get_nc():
    global _NC_CACHE
    if _NC_CACHE is None:
        _NC_CACHE = _build()
    return _NC_CACHE


def _run(colors, densities, depths, trace=False):
    colors_f = np.ascontiguousarray(np.asarray(colors, np.float32).reshape(RAYS, F))
    dens_f = np.ascontiguousarray(np.asarray(densities, np.float32).reshape(RAYS, S))
    dep_f = np.ascontiguousarray(np.asarray(depths, np.float32).reshape(RAYS, S))
    ident = np.eye(P, dtype=np.float32)

    in_maps = []
    for k in range(N_CORES):
        sl = slice(k * RPC, (k + 1) * RPC)
        in_maps.append({
            "colors": colors_f[sl],
            "dens": dens_f[sl],
            "dep": dep_f[sl],
            "ident": ident,
        })
    nc = _get_nc()
    return run_bass_kernel_spmd(nc, in_maps, list(range(N_CORES)), trace=trace)


def kernel(colors, densities, depths, _trace=False, _ret_raw=False):
    res = _run(colors, densities, depths, trace=_trace)
    r = res.results

    weights = np.concatenate([r[k]["weights_o"] for k in range(N_CORES)], axis=0)
    alpha = np.concatenate([r[k]["alpha_o"] for k in range(N_CORES)], axis=0)
    rgbsum = np.concatenate([r[k]["rgbsum_o"] for k in range(N_CORES)], axis=0)
    cdnum2 = np.concatenate([r[k]["cdnum2_o"] for k in range(N_CORES)], axis=0)
    wt = np.concatenate([r[k]["wt_o"] for k in range(N_CORES)], axis=0)
    wbg = np.concatenate([r[k]["wbg_o"] for k in range(N_CORES)], axis=0)

    composite_rgb = (rgbsum - 1.0).reshape(B, R, C)

    depths_np = np.asarray(depths, np.float32)
    with np.errstate(divide="ignore", invalid="ignore"):
        cd = (np.float32(0.5) * cdnum2) / wt
    cd = np.where(np.isnan(cd), np.float32(np.inf), cd)
    cd = np.clip(cd, depths_np.min(), depths_np.max()).astype(np.float32)
    composite_depth = cd.reshape(B, R, 1)

    weights = weights.reshape(B, R, S - 1, 1)
    alpha = alpha.reshape(B, R, S - 1, 1)
    weight_bg = wbg.reshape(B, R, 1, 1)

    out = (composite_rgb, composite_depth, weights, alpha, weight_bg)
    if _ret_raw:
        return out, res
    return out
